# revision 10
# baseline (speedup 1.0000x reference)
"""MAGNN intra-metapath attention aggregation on 8 Trainium2 NeuronCores.

Strategy: edges are sorted by destination node on the host (index-only
preprocessing) and sharded across the 8 cores at 128-node chunk
granularity, so per-destination softmax statistics are core-local and no
collectives are needed.  Chunks are assigned to cores by LPT bin-packing
on edge count and sorted descending inside each core so the SPMD padding
(all cores run the per-slot max block count) stays small.

Math note: the reference computes an edge softmax (segment max, exp,
segment sum) then a weighted scatter-sum.  Because
exp(e - m[dst]) / sum exp(e - m[dst]) == exp(e - C) / sum exp(e - C) for
any constant C, the kernel skips the segment-max pass and uses
w = exp(leaky_relu(er) - 8), whose dynamic range fits fp16.

Device pipeline per 128-edge block (edges on partitions):
  - feat rows arrive PRE-SCALED by attn_r (host fold, fp16):
    ftg[e, h, d] = feat[e, h, d] * attn_r[h, d]
  - er[e,h] = sum_d ftg  -- 5-level pairwise tensor_tensor add tree
    (2x DVE mode) instead of the 1x-only tensor_reduce
  - el = Lrelu(er), w_full[e,h,d] = Exp(el - 8) broadcast over d -- both
    on the scalar engine (the broadcast is fused into the Exp's input AP)
  - whw[:, :256] = ftg * w_full (2x tensor_tensor);
    whw[:, 256:264] = Exp(el - 8)  (denominator columns, scalar engine)
  - sel[e, n] = (iota[n] == dstl[e])  -- tensor_scalar is_equal with the
    per-partition destination as the scalar operand (4x DVE mode)
  - PE: acc[128 nodes, 264] += sel^T @ whw accumulated over the chunk's
    blocks in PSUM
  - epilogue per chunk: den=acc[:,256:264]; out = elu(acc[:,:256]/den
    * (1/attn_r)) with elu(x) = min(exp(x),1)-1+relu(x); fp16 output,
    host upcasts.

The attn_r fold cancels exactly between numerator and denominator except
for fp16 rounding of the stream (which a plain fp16 stream would also
have).  The epilogue multiplies by 1/attn_r to undo the fold on the
output columns.
"""

import json
import sys
import types

import numpy as np

sys.path.insert(0, "/opt/trn_rl_repo")

import jax  # noqa: E402

try:  # persistent compile cache: repeat runs of the same program skip neuronx-cc
    jax.config.update("jax_compilation_cache_dir", "/tmp/jax_cache_magnn")
    jax.config.update("jax_persistent_cache_min_compile_time_secs", 1.0)
    jax.config.update("jax_persistent_cache_min_entry_size_bytes", 0)
except Exception:
    pass

from concourse import bass, mybir  # noqa: E402
from concourse.tile import TileContext  # noqa: E402
from concourse.bass_utils import run_bass_kernel_spmd  # noqa: E402

M_CORES = 8
P = 128  # partitions / edges per block / nodes per chunk
GRP = 16  # blocks processed per instruction group
NEG_SLOPE = 0.01
EXP_SHIFT = 8.0  # constant softmax shift; cancels exactly in num/den

f32 = mybir.dt.float32
f16 = mybir.dt.float16
i32 = mybir.dt.int32


# ---------------------------------------------------------------------------
# BIR fixup: this walrus build rejects instructions carrying more than one
# sync wait ("Too many sync wait commands" in CoreV3 codegen).  Tile's final
# drain aggregates all outstanding semaphore waits onto a single Drain
# instruction.  Splitting the extra waits into standalone EventSemaphore
# instructions on the same engine immediately before is semantically
# identical (each engine executes its instruction stream in order).
# ---------------------------------------------------------------------------

def _split_multi_waits(bir_bytes: bytes) -> bytes:
    js = json.loads(bir_bytes)
    ctr = [0]
    for f in js["functions"]:
        for blk in f["blocks"]:
            out = []
            for inst in blk["instructions"]:
                si = inst.get("sync_info")
                waits = (si or {}).get("on_wait") or []
                if len(waits) > 1:
                    for w in waits[:-1]:
                        ctr[0] += 1
                        out.append({
                            "debug": inst.get("debug", 0),
                            "engine": inst["engine"],
                            "ins": [],
                            "name": f"waitsplit_{ctr[0]}_{inst['name']}",
                            "opcode": "EventSemaphore",
                            "outs": [],
                            "sync_info": {"on_update": [], "on_wait": [w]},
                        })
                    si["on_wait"] = [waits[-1]]
                out.append(inst)
            blk["instructions"] = out
    return json.dumps(js).encode()


def _patch_nc(nc):
    orig = nc.to_json_bytes

    def to_json_bytes(self):
        return _split_multi_waits(orig())

    nc.to_json_bytes = types.MethodType(to_json_bytes, nc)
    return nc


# ---------------------------------------------------------------------------
# Host preprocessing: sort edges by destination, balance 128-node chunks
# across cores, pack each chunk's edges into whole 128-edge blocks.
# ---------------------------------------------------------------------------

def _preprocess(feat, attn_r, metapath_idx, num_nodes):
    feat = np.asarray(feat, dtype=np.float32)
    attn = np.asarray(attn_r, dtype=np.float32).reshape(-1)  # [H*D]
    mp = np.asarray(metapath_idx)
    N = int(num_nodes)
    E, HD = feat.shape
    H = attn_r.shape[-2] if np.asarray(attn_r).ndim == 3 else 8
    D = HD // H

    # attn-folded fp16 stream
    feat16 = (feat * attn[None, :]).astype(np.float16)

    dst = np.asarray(mp[:, 0], dtype=np.int64)
    perm = np.argsort(dst, kind="stable").astype(np.int64)
    ds = dst[perm]

    nchunk_g = -(-N // P)                      # global 128-node chunks
    nchunk = -(-nchunk_g // M_CORES)           # chunk slots per core
    marks = np.minimum(np.arange(nchunk_g + 1) * P, N)
    cb = np.searchsorted(ds, marks)            # chunk edge boundaries
    gcnt = np.diff(cb)                         # [nchunk_g] edges per chunk

    # LPT assignment of global chunks to cores (<= nchunk each), then sort
    # each core's chunks by descending count so slot maxima stay tight.
    order = np.argsort(-gcnt, kind="stable")
    loads = np.zeros(M_CORES, dtype=np.int64)
    slots = np.zeros(M_CORES, dtype=np.int64)
    assign = [[] for _ in range(M_CORES)]
    for k in order:
        m = min((m for m in range(M_CORES) if slots[m] < nchunk),
                key=lambda m: (loads[m], m))
        assign[m].append(int(k))
        loads[m] += gcnt[k]
        slots[m] += 1
    for m in range(M_CORES):
        assign[m].sort(key=lambda k: -gcnt[k])  # descending count
    # chunk_map[m][c] = global chunk id or -1 (dummy)
    chunk_map = np.full((M_CORES, nchunk), -1, dtype=np.int64)
    for m in range(M_CORES):
        chunk_map[m, :len(assign[m])] = assign[m]

    cnt = np.zeros((M_CORES, nchunk), dtype=np.int64)
    for m in range(M_CORES):
        for c in range(nchunk):
            g = chunk_map[m, c]
            if g >= 0:
                cnt[m, c] = gcnt[g]
    Bc = np.maximum(1, -(-cnt // P)).max(axis=0)        # [nchunk]
    T = int(Bc.sum())
    T_pad = (-T) % GRP
    if T_pad:
        Bc[-1] += T_pad                                  # pad last chunk
        T += T_pad
    toff = np.concatenate([[0], np.cumsum(Bc)]).astype(np.int64)

    gidx = np.zeros((M_CORES, T * P), dtype=np.int64)
    dstl = np.full((M_CORES, T, P), -1.0, dtype=np.float32)
    for m in range(M_CORES):
        for c in range(nchunk):
            g = chunk_map[m, c]
            if g < 0:
                continue
            s, e2 = cb[g], cb[g + 1]
            k = int(e2 - s)
            if k == 0:
                continue
            base = int(toff[c]) * P
            gidx[m, base:base + k] = perm[s:e2]
            dstl[m, base // P:(base + k + P - 1) // P].reshape(-1)[:k] = (
                (ds[s:e2] - g * P).astype(np.float32))

    # partition-major stream: featp[p, t*HD:(t+1)*HD] = feat16[gidx[t*P+p]]
    # dstl device layout [P, T]
    plan = {
        "E": E, "HD": HD, "H": H, "D": D, "N": N,
        "nchunk": nchunk, "T": T, "Bc": [int(b) for b in Bc],
        "chunk_map": chunk_map,
    }

    # 1/attn for the epilogue unfold; fp16 when it fits, else fp32
    attn_rec = 1.0 / attn
    rec_dtype = np.float16 if np.abs(attn_rec).max() < 3.0e4 else np.float32
    attn_rec_bc = np.ascontiguousarray(
        np.broadcast_to(attn_rec.astype(rec_dtype), (P, HD)))
    plan["rec_f16"] = rec_dtype == np.float16

    in_maps = []
    for m in range(M_CORES):
        fp = feat16[gidx[m].reshape(T, P)]          # [T, P, HD]
        featp = np.ascontiguousarray(
            fp.transpose(1, 0, 2).reshape(P, T * HD))
        dstlT = np.ascontiguousarray(dstl[m].transpose(1, 0))  # [P, T]
        in_maps.append({"featp": featp, "dstl": dstlT,
                        "attn_rec": attn_rec_bc})
    return plan, in_maps


# ---------------------------------------------------------------------------
# Bass program (SPMD - identical on all 8 cores)
# ---------------------------------------------------------------------------

def _build_nc(plan):
    HD, H, D = plan["HD"], plan["H"], plan["D"]
    nchunk, T, Bc = plan["nchunk"], plan["T"], plan["Bc"]
    NCOLS = HD + H  # matmul rhs: [w*feat | w]
    rec_t = f16 if plan["rec_f16"] else f32

    nc = bass.Bass()
    featp_d = nc.declare_dram_parameter("featp", [P, T * HD], f16,
                                        isOutput=False)
    dstl_d = nc.declare_dram_parameter("dstl", [P, T], f32, isOutput=False)
    arec_d = nc.declare_dram_parameter("attn_rec", [P, HD], rec_t,
                                       isOutput=False)
    out_d = nc.declare_dram_parameter("out", [nchunk * P, HD], f16,
                                      isOutput=True)

    # block index -> (chunk, position-in-chunk)
    c_of, b_of = [], []
    for c in range(nchunk):
        for b in range(Bc[c]):
            c_of.append(c)
            b_of.append(b)

    mult = mybir.AluOpType.mult
    add = mybir.AluOpType.add
    amax = mybir.AluOpType.max
    amin = mybir.AluOpType.min
    is_eq = mybir.AluOpType.is_equal
    AF = mybir.ActivationFunctionType

    with TileContext(nc) as tc:
        with (
            tc.tile_pool(name="const", bufs=1) as p_const,
            tc.tile_pool(name="ft", bufs=3) as p_ft,
            tc.tile_pool(name="tree", bufs=2) as p_tree,
            tc.tile_pool(name="small", bufs=3) as p_small,
            tc.tile_pool(name="wf", bufs=2) as p_wf,
            tc.tile_pool(name="whw", bufs=3) as p_whw,
            tc.tile_pool(name="sel", bufs=12) as p_sel,
            tc.tile_pool(name="psum", bufs=4, space="PSUM") as p_psum,
            tc.tile_pool(name="outp", bufs=3) as p_out,
        ):
            # --- constants / staged index data ---
            dstl_all = p_const.tile([P, T], f32)
            nc.sync.dma_start(out=dstl_all[:], in_=dstl_d[:, :])
            arec = p_const.tile([P, HD], rec_t)
            nc.sync.dma_start(out=arec[:], in_=arec_d[:, :])

            iota_i = p_const.tile([P, P], i32)
            nc.gpsimd.iota(out=iota_i[:], pattern=[[1, P]], base=0,
                           channel_multiplier=0)
            iota_h = p_const.tile([P, P], f16)
            nc.vector.tensor_copy(out=iota_h[:], in_=iota_i[:])

            shift_t = p_const.tile([P, 1], f32)
            nc.vector.memset(shift_t[:], -EXP_SHIFT)

            def epilogue(c, acc):
                # normalize + unfold attn + elu + store one 128-node chunk
                den = p_small.tile([P, H], f32, tag="den")
                nc.vector.tensor_scalar(out=den[:], in0=acc[:, HD:NCOLS],
                                        scalar1=1e-30, scalar2=None, op0=amax)
                rec = p_small.tile([P, H], f32, tag="rec")
                nc.vector.reciprocal(out=rec[:], in_=den[:])
                t1 = p_out.tile([P, HD], f16, tag="t1")
                nc.vector.tensor_tensor(
                    out=t1[:].rearrange("p (h d) -> p h d", d=D),
                    in0=acc[:, 0:HD].rearrange("p (h d) -> p h d", d=D),
                    in1=rec[:, :, None].to_broadcast([P, H, D]),
                    op=mult)
                t2 = p_out.tile([P, HD], f16, tag="t2")
                nc.vector.tensor_tensor(out=t2[:], in0=t1[:], in1=arec[:],
                                        op=mult)
                # elu(x) = (min(exp(x),1) - 1) + relu(x)
                e1 = p_out.tile([P, HD], f16, tag="e1")
                nc.scalar.activation(out=e1[:], in_=t2[:], func=AF.Exp)
                nc.vector.tensor_scalar(out=e1[:], in0=e1[:],
                                        scalar1=1.0, scalar2=-1.0,
                                        op0=amin, op1=add)
                osb = p_out.tile([P, HD], f16, tag="osb")
                nc.vector.scalar_tensor_tensor(
                    out=osb[:], in0=t2[:], scalar=0.0, in1=e1[:],
                    op0=amax, op1=add)
                oeng = nc.sync if c % 2 == 0 else nc.scalar
                oeng.dma_start(out=out_d[c * P:(c + 1) * P, :], in_=osb[:])

            # --- main loop over groups of GRP blocks ---
            acc = None
            for t0 in range(0, T, GRP):
                g = GRP
                ftg = p_ft.tile([P, g * HD], f16, tag="ft")
                # rotate across the three DMA rings (SP / Activation
                # HWDGE + gpsimd SWDGE) so the feature stream doesn't
                # serialize on one ring
                dma_eng = (nc.sync, nc.scalar, nc.gpsimd)[(t0 // GRP) % 3]
                dma_eng.dma_start(out=ftg[:],
                                  in_=featp_d[:, t0 * HD:(t0 + g) * HD])
                ft4 = ftg[:].rearrange("p (g h d) -> p g h d", h=H, d=D)

                # er = sum_d ftg : pairwise add tree (fp16 until the last add)
                t16 = p_tree.tile([P, g * H * 16], f16, tag="t16")
                v16 = t16[:].rearrange("p (g h d) -> p g h d", h=H, d=16)
                nc.vector.tensor_tensor(out=v16, in0=ft4[:, :, :, 0:16],
                                        in1=ft4[:, :, :, 16:32], op=add)
                t8 = p_tree.tile([P, g * H * 8], f16, tag="t8")
                v8 = t8[:].rearrange("p (g h d) -> p g h d", h=H, d=8)
                nc.vector.tensor_tensor(out=v8, in0=v16[:, :, :, 0:8],
                                        in1=v16[:, :, :, 8:16], op=add)
                t4 = p_tree.tile([P, g * H * 4], f16, tag="t4")
                v4 = t4[:].rearrange("p (g h d) -> p g h d", h=H, d=4)
                nc.vector.tensor_tensor(out=v4, in0=v8[:, :, :, 0:4],
                                        in1=v8[:, :, :, 4:8], op=add)
                t2t = p_tree.tile([P, g * H * 2], f16, tag="t2")
                v2 = t2t[:].rearrange("p (g h d) -> p g h d", h=H, d=2)
                nc.vector.tensor_tensor(out=v2, in0=v4[:, :, :, 0:2],
                                        in1=v4[:, :, :, 2:4], op=add)
                er = p_small.tile([P, g * H], f32, tag="er")
                ve = er[:].rearrange("p (g h) -> p g h", h=H)
                nc.vector.tensor_tensor(out=ve[:, :, :, None],
                                        in0=v2[:, :, :, 0:1],
                                        in1=v2[:, :, :, 1:2], op=add)

                # el = lrelu(er); w = exp(el - SHIFT)
                el = p_small.tile([P, g * H], f32, tag="el")
                nc.scalar.activation(out=el[:], in_=er[:], func=AF.Lrelu,
                                     alpha=NEG_SLOPE)
                whw = p_whw.tile([P, g * NCOLS], f16, tag="whw")
                whw3 = whw[:].rearrange("p (g c) -> p g c", c=NCOLS)
                el3 = el[:].rearrange("p (g h) -> p g h", h=H)
                # denominator columns: w
                nc.scalar.activation(out=whw3[:, :, HD:NCOLS], in_=el3,
                                     func=AF.Exp, bias=shift_t[:])
                # w broadcast over d=8 only (quarter resolution): 4x less
                # scalar-engine work; the multiply reads it 4x instead
                DQ = 8
                wf = p_wf.tile([P, g * H * DQ], f16, tag="wf")
                wf4 = wf[:].rearrange("p (g h d) -> p g h d", h=H, d=DQ)
                nc.scalar.activation(
                    out=wf4, in_=el3[:, :, :, None].to_broadcast([P, g, H, DQ]),
                    func=AF.Exp, bias=shift_t[:])
                # whw[:, :256] = ftg * w  (4 dense 2x tensor_tensors)
                whw4 = whw3[:, :, 0:HD].rearrange("p g (h d) -> p g h d", d=D)
                for q in range(D // DQ):
                    nc.vector.tensor_tensor(
                        out=whw4[:, :, :, q * DQ:(q + 1) * DQ],
                        in0=ft4[:, :, :, q * DQ:(q + 1) * DQ],
                        in1=wf4, op=mult)

                for j in range(g):
                    t = t0 + j
                    c, b = c_of[t], b_of[t]
                    sel = p_sel.tile([P, P], f16, tag="sel")
                    nc.vector.tensor_scalar(
                        out=sel[:], in0=iota_h[:],
                        scalar1=dstl_all[:, t:t + 1], scalar2=None,
                        op0=is_eq)
                    if b == 0:
                        acc = p_psum.tile([P, NCOLS], f32, space="PSUM",
                                          tag="acc")
                    nc.tensor.matmul(
                        out=acc[:], lhsT=sel[:],
                        rhs=whw[:, j * NCOLS:(j + 1) * NCOLS],
                        start=(b == 0), stop=(b == Bc[c] - 1))
                    if b == Bc[c] - 1:
                        epilogue(c, acc)

    _patch_nc(nc)
    return nc


# ---------------------------------------------------------------------------
# public entry point
# ---------------------------------------------------------------------------

def prepare(feat, attn_r, metapath_idx, num_nodes):
    plan, in_maps = _preprocess(feat, attn_r, metapath_idx, num_nodes)
    nc = _build_nc(plan)
    return plan, in_maps, nc


def assemble(plan, results):
    N, HD, nchunk = plan["N"], plan["HD"], plan["nchunk"]
    chunk_map = plan["chunk_map"]
    out = np.zeros((N, HD), dtype=np.float32)
    for m in range(M_CORES):
        res = np.asarray(results[m]["out"], dtype=np.float32)
        for c in range(nchunk):
            g = int(chunk_map[m, c])
            if g < 0:
                continue
            lo = g * P
            hi = min(lo + P, N)
            out[lo:hi] = res[c * P:c * P + (hi - lo)]
    return out


def kernel(feat, attn_r, metapath_idx, num_nodes):
    plan, in_maps, nc = prepare(feat, attn_r, metapath_idx, num_nodes)
    res = run_bass_kernel_spmd(nc, in_maps, list(range(M_CORES)))
    return assemble(plan, res.results)
